# revision 2
# baseline (speedup 1.0000x reference)
"""Polynomial features (degree 2) + linear layer, distributed over 8 TRN2 cores.

reference: A = [x, {x_i*x_j for i<=j}] (8384 coeffs); out = A @ W.T + b.

Pair enumeration by circular distance class s in 0..64: class s, lane p
-> unordered pair {p, (p+s)%128}; host ships rotated copies of x^T so
every class is an aligned op of two rotations (rot d: row p = feature
(p+d)%128).

The per-class work is split across four engines to balance the machine
(DVE elementwise is the scarcest resource):
  - s=0 (squares x_p^2): ACT Square, straight from SBUF.
  - DVE classes (families 0-4, s=1..40): bf16 tensor_mul of two
    rotations (2x perf mode), as one multi-class strided op per family.
  - HOSTQ classes (s=41..54 by default): x_u*x_v = 0.5*q^2 - corr where
    q = x_u + x_v; the host ships q^2 pre-squared in bf16 (pure DMA,
    no on-chip elementwise at all).
  - TRICK classes (s=55..64 by default): q computed on the Tensor
    engine (two identity matmuls accumulating into PSUM), squared by
    ACT (PSUM->SBUF); processed in pairs sharing a 2-bank PSUM tile.
  For HOSTQ+TRICK classes the class weight block is halved and the
  -0.5*W_s*(x_u^2+x_v^2) corrections fold into the squares block
  (whose rhs is the ACT x^2 tile) -- free.
  - 66 K=128 matmuls per 512-batch tile accumulate into PSUM [128, 512]
    (even classes + linear -> array cols 0-63 / psum partitions 0:64,
    odd -> cols 64-127) using 2x column tiling, so even/odd matmuls run
    concurrently on the PE.
  - one ACT evacuates the full PSUM tile (bias zero-padded on the odd
    half); the halves are summed on the HOST after gathering (avoids
    the slow software-DGE accumulate DMA).
  - TPB instructions have a single sync-wait slot; _split_multiwaits()
    hoists extra Tile-emitted waits onto injected same-engine NOPs.
"""

import numpy as np
import ml_dtypes
import os

INPUT_DIM = 128
OUTPUT_DIM = 64
BATCH = 32768
N_CORES = 8
B_CORE = BATCH // N_CORES  # 4096
TILE_B = 512
N_TILES = B_CORE // TILE_B  # 8

N_TRICK = int(os.environ.get("K_TRICK", "10"))  # PE-add + ACT-square classes
N_HOSTQ = int(os.environ.get("K_HOSTQ", "14"))  # host-shipped q^2 classes
assert N_TRICK % 2 == 0
TRICK_CLASSES = list(range(65 - N_TRICK, 65))
HOSTQ_CLASSES = list(range(65 - N_TRICK - N_HOSTQ, 65 - N_TRICK))
N_DVE = 64 - N_TRICK - N_HOSTQ  # classes 1..N_DVE on the DVE


def _class_ops():
    """(a, b) rotation pair per distance class s=0..64 with b - a = s."""
    ops = []
    for s in range(65):
        if s <= 8:
            a, b = 0, s
        else:
            k = (s - 1) // 8  # 1..7
            anchor = 8 * k + 8
            a, b = anchor - s, anchor
        assert b - a == s, (s, a, b)
        ops.append((a, b))
    return ops


CLASS_OPS = _class_ops()

# rotations needed on-device: by DVE classes and trick add-matmuls only
# (hostq classes are materialized host-side)
_need = {0}
for s in range(1, 65):
    if s <= N_DVE or s in TRICK_CLASSES:
        a, b = CLASS_OPS[s]
        _need.add(a)
        _need.add(b)
ROT_SET = sorted(_need)
N_ROT = len(ROT_SET)
ROT_IDX = {d: i for i, d in enumerate(ROT_SET)}

# DVE groups: per-family runs of classes 1..N_DVE (family f = 8f+1..8f+8)
DVE_GROUPS = []
for f in range(8):
    g = [s for s in range(8 * f + 1, 8 * f + 9) if s <= N_DVE]
    if g:
        DVE_GROUPS.append(g)


def _build_device_weights(W, b):
    """Permute W [64, 8384] into the device K-block layout.

    Returns w_packed [128, 66*64]: block j (j=0 linear, j=1+s class s)
    lives at free columns [j*64, (j+1)*64), partition p = K row p.
    Class s row p -> pair {p, (p+s)%128}; s=64 rows p>=64 are zeroed dups.
    Trick/hostq classes: block halved; -0.5*W*(x_u^2 + x_v^2) corrections
    accumulate into the squares block (1+0), whose rhs is x_p^2 on lane p.
    """
    W = np.asarray(W, np.float32)
    n = INPUT_DIM
    pair_off = {}
    c = 0
    for i in range(n):
        for j in range(i, n):
            pair_off[(i, j)] = c
            c += 1
    assert c == 8256

    Wd = np.zeros((66, 128, OUTPUT_DIM), np.float32)
    Wd[0] = W[:, 0:128].T  # linear block
    seen = set()
    for s in range(65):
        a, _bb = CLASS_OPS[s]
        for p in range(128):
            u = (p + a) % 128
            v = (p + a + s) % 128
            i, j = (u, v) if u <= v else (v, u)
            if (i, j) in seen:
                continue  # duplicate lane (s=64 second half)
            seen.add((i, j))
            Wd[1 + s, p] = W[:, 128 + pair_off[(i, j)]]
    assert len(seen) == 8256, len(seen)

    for s in TRICK_CLASSES + HOSTQ_CLASSES:
        a, _bb = CLASS_OPS[s]
        Wd[1 + s] *= 0.5
        for p in range(128):
            u = (p + a) % 128
            v = (p + a + s) % 128
            Wd[1, u] -= Wd[1 + s, p]
            Wd[1, v] -= Wd[1 + s, p]

    w_packed = np.ascontiguousarray(
        Wd.transpose(1, 0, 2).reshape(128, 66 * OUTPUT_DIM)
    ).astype(ml_dtypes.bfloat16)
    return w_packed, np.asarray(b, np.float32)


def _split_multiwaits(nc, mybir):
    """TPB instructions have one sync-wait slot; hoist extras onto NOPs."""
    import bass_rust

    n_split = 0
    for fn in nc.m.functions:
        for bb in fn.blocks:
            out = []
            changed = False
            for inst in bb.instructions:
                si = getattr(inst, "sync_info", None)
                if si is not None and si.on_wait and len(si.on_wait) > 1:
                    for w in si.on_wait[:-1]:
                        n_split += 1
                        nop = bass_rust.InstNoOp(
                            name=f"I-mw{n_split}",
                            engine=inst.engine,
                            ins=[],
                            outs=[],
                            sync_info=mybir.SyncInfo(on_wait=[w], on_update=[]),
                            bass_nofuse=True,
                        )
                        out.append(nop)
                    inst.sync_info = mybir.SyncInfo(
                        on_wait=[si.on_wait[-1]], on_update=si.on_update
                    )
                    changed = True
                out.append(inst)
            if changed:
                bb.instructions = out
    return n_split


def build(x, W, b):
    """Build the Bass graph and per-core input maps. Returns (nc, in_maps)."""
    import concourse.bass as bass
    import concourse.mybir as mybir
    from concourse import tile

    bf16 = mybir.dt.bfloat16
    f32 = mybir.dt.float32

    # ---- host preprocessing ----
    xT = np.ascontiguousarray(np.asarray(x, np.float32).T)  # [128, 32768] f32
    xTb = xT.astype(ml_dtypes.bfloat16)
    xall = np.stack([np.roll(xTb, -d, axis=0) for d in ROT_SET], axis=1)
    # hostq: q^2 = (x_u + x_v)^2 in f32, shipped as bf16
    if N_HOSTQ:
        q2h = np.stack(
            [
                (np.roll(xT, -CLASS_OPS[s][0], axis=0)
                 + np.roll(xT, -CLASS_OPS[s][1], axis=0)) ** 2
                for s in HOSTQ_CLASSES
            ],
            axis=1,
        ).astype(ml_dtypes.bfloat16)  # [128, N_HOSTQ, 32768]
    w_packed, bias = _build_device_weights(W, b)
    ident = np.eye(128, dtype=ml_dtypes.bfloat16)

    # ---- device graph ----
    nc = bass.Bass()
    x_in = nc.declare_dram_parameter(
        "xall", [N_TILES, 128, N_ROT, TILE_B], bf16, isOutput=False
    )
    if N_HOSTQ:
        q_in = nc.declare_dram_parameter(
            "q2all", [N_TILES, 128, N_HOSTQ, TILE_B], bf16, isOutput=False
        )
    w_in = nc.declare_dram_parameter("Wd", [128, 66 * 64], bf16, isOutput=False)
    b_in = nc.declare_dram_parameter("bias", [128, 1], f32, isOutput=False)
    id_in = nc.declare_dram_parameter("ident", [128, 128], bf16, isOutput=False)
    out_ext = nc.declare_dram_parameter(
        "outT", [128, B_CORE], f32, isOutput=True
    )

    def rot_group_ap(xrt, classes):
        """[128, len(classes), TILE_B] APs (in0, in1)."""
        m = len(classes)
        us = [ROT_IDX[CLASS_OPS[s][0]] for s in classes]
        vs = [ROT_IDX[CLASS_OPS[s][1]] for s in classes]

        def mk(idx):
            if all(i == idx[0] for i in idx):
                return xrt[:, idx[0] : idx[0] + 1, :].to_broadcast(
                    [128, m, TILE_B]
                )
            d = idx[1] - idx[0]
            assert all(idx[j + 1] - idx[j] == d for j in range(m - 1)), idx
            return xrt[:, idx[0] :: d, :][:, 0:m, :]

        return mk(us), mk(vs)

    with tile.TileContext(nc) as tc:
        with (
            tc.tile_pool(name="consts", bufs=1) as consts,
            tc.tile_pool(name="xc", bufs=4) as xcp,
            tc.tile_pool(name="qc", bufs=3) as qcp,
            tc.tile_pool(name="prod", bufs=8) as prodp,
            tc.tile_pool(name="sq", bufs=N_TRICK + 1) as sqp,
            tc.tile_pool(name="x2p", bufs=3) as x2p,
            tc.tile_pool(name="outp", bufs=3) as outp,
            tc.tile_pool(name="psum", bufs=2, space="PSUM") as psump,
            tc.tile_pool(name="qpsum", bufs=3, space="PSUM") as qpsump,
        ):
            xc_tiles = [None] * (N_TILES + 3)
            qc_tiles = [None] * (N_TILES + 3)
            w_sb = consts.tile([128, 66 * 64], bf16)
            b_sb = consts.tile([128, 1], f32)
            id_sb = consts.tile([128, 128], bf16)

            def load_xc(t):
                if t >= N_TILES:
                    return
                xt = xcp.tile([128, N_ROT, TILE_B], bf16, tag="xc", name="xc_t")
                # two chunks so early consumers start sooner (family 0
                # needs rots 0..8 only)
                h = min(9, N_ROT)
                nc.sync.dma_start(xt[:, 0:h, :], x_in[t][:, 0:h, :])
                nc.sync.dma_start(xt[:, h:N_ROT, :], x_in[t][:, h:N_ROT, :])
                xc_tiles[t] = xt
                if N_HOSTQ:
                    qt = qcp.tile(
                        [128, N_HOSTQ, TILE_B], bf16, tag="qc", name="qc_t"
                    )
                    nc.sync.dma_start(qt[:], q_in[t][:])
                    qc_tiles[t] = qt

            def trick_phase(t):
                """PE adds (identity matmuls) + ACT squares for tile t's
                trick classes, plus tile t's x^2; issued one tile ahead so
                the q2 tiles are ready before their class matmuls. Classes
                are processed in pairs: both sums land in one 2-bank PSUM
                tile, squared by a single ACT op."""
                if t >= N_TILES:
                    return None, None
                xrt = xc_tiles[t]
                q2_tiles = {}
                for i in range(0, N_TRICK, 2):
                    pair = TRICK_CLASSES[i : i + 2]
                    qps = qpsump.tile(
                        [128, 2, TILE_B], f32, tag="q", name="q_ps"
                    )
                    for j, s in enumerate(pair):
                        a, bb = CLASS_OPS[s]
                        nc.tensor.matmul(
                            qps[:, j, :],
                            id_sb[:],
                            xrt[:, ROT_IDX[a], :],
                            start=True,
                            stop=False,
                        )
                        nc.tensor.matmul(
                            qps[:, j, :],
                            id_sb[:],
                            xrt[:, ROT_IDX[bb], :],
                            start=False,
                            stop=True,
                        )
                    q2 = sqp.tile(
                        [128, 2, TILE_B], bf16, tag="q2", name="q2_t"
                    )
                    nc.scalar.activation(
                        q2[:], qps[:], mybir.ActivationFunctionType.Square
                    )
                    for j, s in enumerate(pair):
                        q2_tiles[s] = q2[:, j, :]
                # squares class (s=0): x_p^2 on ACT straight from SBUF
                x2 = x2p.tile([128, TILE_B], bf16, tag="x2", name="x2_t")
                nc.scalar.activation(
                    x2[:], xrt[:, 0, :], mybir.ActivationFunctionType.Square
                )
                return q2_tiles, x2

            # xc[0] first (it gates the whole pipeline), then the identity
            # (gates the first add-matmuls), then weights
            load_xc(0)
            nc.sync.dma_start(id_sb[:], id_in[:])
            load_xc(1)
            nc.sync.dma_start(w_sb[:], w_in[:])
            nc.sync.dma_start(b_sb[:], b_in[:])
            load_xc(2)
            trick = [None] * (N_TILES + 1)
            trick[0] = trick_phase(0)
            for t in range(N_TILES):
                load_xc(t + 3)
                xrt = xc_tiles[t]
                qrt = qc_tiles[t]
                trick[t + 1] = trick_phase(t + 1)
                q2_tiles, x2 = trick[t]

                # DVE product groups
                prod_views = {}  # class s -> AP of its product row
                for k, classes in enumerate(DVE_GROUPS):
                    m = len(classes)
                    p_t = prodp.tile(
                        [128, m, TILE_B], bf16, tag="prod" + str(m), name="p_t"
                    )
                    in0, in1 = rot_group_ap(xrt, classes)
                    nc.vector.tensor_mul(p_t[:], in0, in1)
                    for j, s in enumerate(classes):
                        prod_views[s] = p_t[:, j, :]

                # acc halves: even classes + linear -> partitions 0:64
                # (array cols 0-63), odd classes -> partitions 64:128
                acc = psump.tile([128, TILE_B], f32, name="acc")
                nc.tensor.matmul(
                    acc[0:64, :],
                    w_sb[:, 0:64],
                    xrt[:, 0, :],
                    start=True,
                    stop=False,
                    tile_position=(0, 0),
                )
                first_odd = True
                for s in range(65):
                    if s == 0:
                        rhs = x2[:]
                    elif s in q2_tiles:
                        rhs = q2_tiles[s]
                    elif s in prod_views:
                        rhs = prod_views[s]
                    else:
                        rhs = qrt[:, s - HOSTQ_CLASSES[0], :]
                    half = s % 2
                    blk = 1 + s
                    is_last_even = s == 64
                    is_last_odd = s == 63
                    nc.tensor.matmul(
                        acc[64 * half : 64 * half + 64, :],
                        w_sb[:, blk * 64 : (blk + 1) * 64],
                        rhs,
                        start=(half == 1 and first_odd),
                        stop=(is_last_even or is_last_odd),
                        tile_position=(0, 64 * half),
                    )
                    if half == 1:
                        first_odd = False

                # one ACT evacuates both PSUM halves (bias zero-padded on
                # the odd half); halves are summed host-side after gather
                o_t = outp.tile([128, TILE_B], f32, tag="o", name="o_t")
                nc.scalar.activation(
                    o_t[:],
                    acc[:],
                    mybir.ActivationFunctionType.Identity,
                    bias=b_sb[:, 0:1],
                )
                bs = slice(t * TILE_B, (t + 1) * TILE_B)
                nc.scalar.dma_start(out_ext[:, bs], o_t[:])

    _split_multiwaits(nc, mybir)

    # ---- per-core input maps ----
    in_maps = []
    for c in range(N_CORES):
        csl = slice(c * B_CORE, (c + 1) * B_CORE)
        cs = xall[:, :, csl]  # [128, N_ROT, 4096]
        xtiles = np.ascontiguousarray(
            cs.reshape(128, N_ROT, N_TILES, TILE_B).transpose(2, 0, 1, 3)
        )
        bias128 = np.zeros((128, 1), np.float32)
        bias128[:OUTPUT_DIM, 0] = bias
        m = {
            "xall": xtiles,
            "Wd": w_packed,
            "bias": bias128,
            "ident": np.ascontiguousarray(ident),
        }
        if N_HOSTQ:
            qs = q2h[:, :, csl]
            m["q2all"] = np.ascontiguousarray(
                qs.reshape(128, N_HOSTQ, N_TILES, TILE_B).transpose(2, 0, 1, 3)
            )
        in_maps.append(m)
    return nc, in_maps


def assemble(results):
    """Gather per-core outputs into the full [BATCH, 64] array (summing
    the even/odd PSUM halves host-side)."""
    outs = []
    for r in results:
        o = np.asarray(r["outT"], np.float32)  # [128, B_CORE]
        outs.append((o[0:OUTPUT_DIM] + o[OUTPUT_DIM:128]).T)
    return np.concatenate(outs, axis=0)


def kernel(x, W, b, indices_0, indices_1):
    from concourse.bass_utils import run_bass_kernel_spmd

    nc, in_maps = build(x, W, b)
    res = run_bass_kernel_spmd(nc, in_maps, list(range(N_CORES))).results
    return assemble(res)


# revision 3
# speedup vs baseline: 1.0660x; 1.0660x over previous
"""Polynomial features (degree 2) + linear layer, distributed over 8 TRN2 cores.

reference: A = [x, {x_i*x_j for i<=j}] (8384 coeffs); out = A @ W.T + b.

Pair enumeration by circular distance class s in 0..64: class s, lane p
-> unordered pair {p, (p+s)%128}; host ships rotated copies of x^T so
every class is an aligned op of two rotations (rot d: row p = feature
(p+d)%128).

The per-class work is split across four engines to balance the machine
(DVE elementwise is the scarcest resource):
  - s=0 (squares x_p^2): ACT Square, straight from SBUF.
  - DVE classes (families 0-4, s=1..40): bf16 tensor_mul of two
    rotations (2x perf mode), as one multi-class strided op per family.
  - HOSTQ classes (s=41..54 by default): x_u*x_v = 0.5*q^2 - corr where
    q = x_u + x_v; the host ships q^2 pre-squared in bf16 (pure DMA,
    no on-chip elementwise at all).
  - TRICK classes (s=55..64 by default): q computed on the Tensor
    engine (two identity matmuls accumulating into PSUM), squared by
    ACT (PSUM->SBUF); processed in pairs sharing a 2-bank PSUM tile.
  For HOSTQ+TRICK classes the class weight block is halved and the
  -0.5*W_s*(x_u^2+x_v^2) corrections fold into the squares block
  (whose rhs is the ACT x^2 tile) -- free.
  - 66 K=128 matmuls per 512-batch tile accumulate into PSUM [128, 512]
    (even classes + linear -> array cols 0-63 / psum partitions 0:64,
    odd -> cols 64-127) using 2x column tiling, so even/odd matmuls run
    concurrently on the PE.
  - one ACT evacuates the full PSUM tile (bias zero-padded on the odd
    half); the halves are summed on the HOST after gathering (avoids
    the slow software-DGE accumulate DMA).
  - TPB instructions have a single sync-wait slot; _split_multiwaits()
    hoists extra Tile-emitted waits onto injected same-engine NOPs.
"""

import numpy as np
import ml_dtypes
import os

INPUT_DIM = 128
OUTPUT_DIM = 64
BATCH = 32768
N_CORES = 8
B_CORE = BATCH // N_CORES  # 4096
TILE_B = 512
N_TILES = B_CORE // TILE_B  # 8

N_TRICK = int(os.environ.get("K_TRICK", "8"))  # PE-add + ACT-square classes
N_HOSTQ = int(os.environ.get("K_HOSTQ", "16"))  # host-shipped q^2 classes
assert N_TRICK % 2 == 0
TRICK_CLASSES = list(range(65 - N_TRICK, 65))
HOSTQ_CLASSES = list(range(65 - N_TRICK - N_HOSTQ, 65 - N_TRICK))
N_DVE = 64 - N_TRICK - N_HOSTQ  # classes 1..N_DVE on the DVE


def _class_ops():
    """(a, b) rotation pair per distance class s=0..64 with b - a = s."""
    ops = []
    for s in range(65):
        if s <= 8:
            a, b = 0, s
        else:
            k = (s - 1) // 8  # 1..7
            anchor = 8 * k + 8
            a, b = anchor - s, anchor
        assert b - a == s, (s, a, b)
        ops.append((a, b))
    return ops


CLASS_OPS = _class_ops()

# rotations needed on-device: by DVE classes and trick add-matmuls only
# (hostq classes are materialized host-side). Order: low rots, then trick
# anchors, then DVE anchors -- so the first xc DMA chunk (rots [0:N_EARLY])
# already unblocks the trick add-matmuls and the family-0 product op.
_need = {0}
_trick_anchors = set()
for s in range(1, 65):
    if s <= N_DVE or s in TRICK_CLASSES:
        a, b = CLASS_OPS[s]
        _need.add(a)
        _need.add(b)
        if s in TRICK_CLASSES:
            _trick_anchors.update(r for r in (a, b) if r > 8)
ROT_SET = (
    sorted(r for r in _need if r <= 8)
    + sorted(_trick_anchors)
    + sorted(r for r in _need if r > 8 and r not in _trick_anchors)
)
N_ROT = len(ROT_SET)
N_EARLY = sum(1 for r in _need if r <= 8) + len(_trick_anchors)
ROT_IDX = {d: i for i, d in enumerate(ROT_SET)}

# DVE groups: per-family runs of classes 1..N_DVE (family f = 8f+1..8f+8)
DVE_GROUPS = []
for f in range(8):
    g = [s for s in range(8 * f + 1, 8 * f + 9) if s <= N_DVE]
    if g:
        DVE_GROUPS.append(g)


def _build_device_weights(W, b):
    """Permute W [64, 8384] into the device K-block layout.

    Returns w_packed [128, 66*64]: block j (j=0 linear, j=1+s class s)
    lives at free columns [j*64, (j+1)*64), partition p = K row p.
    Class s row p -> pair {p, (p+s)%128}; s=64 rows p>=64 are zeroed dups.
    Trick/hostq classes: block halved; -0.5*W*(x_u^2 + x_v^2) corrections
    accumulate into the squares block (1+0), whose rhs is x_p^2 on lane p.
    """
    W = np.asarray(W, np.float32)
    n = INPUT_DIM
    pair_off = {}
    c = 0
    for i in range(n):
        for j in range(i, n):
            pair_off[(i, j)] = c
            c += 1
    assert c == 8256

    Wd = np.zeros((66, 128, OUTPUT_DIM), np.float32)
    Wd[0] = W[:, 0:128].T  # linear block
    seen = set()
    for s in range(65):
        a, _bb = CLASS_OPS[s]
        for p in range(128):
            u = (p + a) % 128
            v = (p + a + s) % 128
            i, j = (u, v) if u <= v else (v, u)
            if (i, j) in seen:
                continue  # duplicate lane (s=64 second half)
            seen.add((i, j))
            Wd[1 + s, p] = W[:, 128 + pair_off[(i, j)]]
    assert len(seen) == 8256, len(seen)

    for s in TRICK_CLASSES + HOSTQ_CLASSES:
        a, _bb = CLASS_OPS[s]
        Wd[1 + s] *= 0.5
        for p in range(128):
            u = (p + a) % 128
            v = (p + a + s) % 128
            Wd[1, u] -= Wd[1 + s, p]
            Wd[1, v] -= Wd[1 + s, p]

    w_packed = np.ascontiguousarray(
        Wd.transpose(1, 0, 2).reshape(128, 66 * OUTPUT_DIM)
    ).astype(ml_dtypes.bfloat16)
    return w_packed, np.asarray(b, np.float32)


def _split_multiwaits(nc, mybir):
    """TPB instructions have one sync-wait slot; hoist extras onto NOPs."""
    import bass_rust

    n_split = 0
    for fn in nc.m.functions:
        for bb in fn.blocks:
            out = []
            changed = False
            for inst in bb.instructions:
                si = getattr(inst, "sync_info", None)
                if si is not None and si.on_wait and len(si.on_wait) > 1:
                    for w in si.on_wait[:-1]:
                        n_split += 1
                        nop = bass_rust.InstNoOp(
                            name=f"I-mw{n_split}",
                            engine=inst.engine,
                            ins=[],
                            outs=[],
                            sync_info=mybir.SyncInfo(on_wait=[w], on_update=[]),
                            bass_nofuse=True,
                        )
                        out.append(nop)
                    inst.sync_info = mybir.SyncInfo(
                        on_wait=[si.on_wait[-1]], on_update=si.on_update
                    )
                    changed = True
                out.append(inst)
            if changed:
                bb.instructions = out
    return n_split


def build(x, W, b):
    """Build the Bass graph and per-core input maps. Returns (nc, in_maps)."""
    import concourse.bass as bass
    import concourse.mybir as mybir
    from concourse import tile

    bf16 = mybir.dt.bfloat16
    f32 = mybir.dt.float32

    # ---- host preprocessing ----
    xT = np.ascontiguousarray(np.asarray(x, np.float32).T)  # [128, 32768] f32
    xTb = xT.astype(ml_dtypes.bfloat16)
    xall = np.stack([np.roll(xTb, -d, axis=0) for d in ROT_SET], axis=1)
    # hostq: q^2 = (x_u + x_v)^2 in f32, shipped as bf16
    if N_HOSTQ:
        q2h = np.stack(
            [
                (np.roll(xT, -CLASS_OPS[s][0], axis=0)
                 + np.roll(xT, -CLASS_OPS[s][1], axis=0)) ** 2
                for s in HOSTQ_CLASSES
            ],
            axis=1,
        ).astype(ml_dtypes.bfloat16)  # [128, N_HOSTQ, 32768]
    w_packed, bias = _build_device_weights(W, b)
    ident = np.eye(128, dtype=ml_dtypes.bfloat16)

    # ---- device graph ----
    nc = bass.Bass()
    x_in = nc.declare_dram_parameter(
        "xall", [N_TILES, 128, N_ROT, TILE_B], bf16, isOutput=False
    )
    if N_HOSTQ:
        q_in = nc.declare_dram_parameter(
            "q2all", [N_TILES, 128, N_HOSTQ, TILE_B], bf16, isOutput=False
        )
    w_in = nc.declare_dram_parameter("Wd", [128, 66 * 64], bf16, isOutput=False)
    b_in = nc.declare_dram_parameter("bias", [128, 1], f32, isOutput=False)
    id_in = nc.declare_dram_parameter("ident", [128, 128], bf16, isOutput=False)
    out_ext = nc.declare_dram_parameter(
        "outT", [128, B_CORE], f32, isOutput=True
    )

    def rot_group_ap(xrt, classes):
        """[128, len(classes), TILE_B] APs (in0, in1)."""
        m = len(classes)
        us = [ROT_IDX[CLASS_OPS[s][0]] for s in classes]
        vs = [ROT_IDX[CLASS_OPS[s][1]] for s in classes]

        def mk(idx):
            if all(i == idx[0] for i in idx):
                return xrt[:, idx[0] : idx[0] + 1, :].to_broadcast(
                    [128, m, TILE_B]
                )
            d = idx[1] - idx[0]
            assert all(idx[j + 1] - idx[j] == d for j in range(m - 1)), idx
            return xrt[:, idx[0] :: d, :][:, 0:m, :]

        return mk(us), mk(vs)

    with tile.TileContext(nc) as tc:
        with (
            tc.tile_pool(name="consts", bufs=1) as consts,
            tc.tile_pool(name="xc", bufs=4) as xcp,
            tc.tile_pool(name="qc", bufs=3) as qcp,
            tc.tile_pool(name="prod", bufs=8) as prodp,
            tc.tile_pool(name="sq", bufs=N_TRICK + 1) as sqp,
            tc.tile_pool(name="x2p", bufs=3) as x2p,
            tc.tile_pool(name="outp", bufs=3) as outp,
            tc.tile_pool(name="psum", bufs=2, space="PSUM") as psump,
            tc.tile_pool(name="qpsum", bufs=3, space="PSUM") as qpsump,
        ):
            xc_tiles = [None] * (N_TILES + 3)
            qc_tiles = [None] * (N_TILES + 3)
            w_sb = consts.tile([128, 66 * 64], bf16)
            b_sb = consts.tile([128, 1], f32)
            id_sb = consts.tile([128, 128], bf16)

            def load_xc(t):
                if t >= N_TILES:
                    return
                xt = xcp.tile([128, N_ROT, TILE_B], bf16, tag="xc", name="xc_t")
                # two chunks so early consumers (trick add-matmuls and the
                # family-0 product op) start sooner
                h = min(N_EARLY, N_ROT)
                nc.sync.dma_start(xt[:, 0:h, :], x_in[t][:, 0:h, :])
                if h < N_ROT:
                    nc.sync.dma_start(xt[:, h:N_ROT, :], x_in[t][:, h:N_ROT, :])
                xc_tiles[t] = xt
                if N_HOSTQ:
                    qt = qcp.tile(
                        [128, N_HOSTQ, TILE_B], bf16, tag="qc", name="qc_t"
                    )
                    nc.sync.dma_start(qt[:], q_in[t][:])
                    qc_tiles[t] = qt

            def trick_phase(t):
                """PE adds (identity matmuls) + ACT squares for tile t's
                trick classes, plus tile t's x^2; issued one tile ahead so
                the q2 tiles are ready before their class matmuls. Classes
                are processed in pairs: both sums land in one 2-bank PSUM
                tile, squared by a single ACT op."""
                if t >= N_TILES:
                    return None, None
                xrt = xc_tiles[t]
                q2_tiles = {}
                for i in range(0, N_TRICK, 2):
                    pair = TRICK_CLASSES[i : i + 2]
                    qps = qpsump.tile(
                        [128, 2, TILE_B], f32, tag="q", name="q_ps"
                    )
                    for j, s in enumerate(pair):
                        a, bb = CLASS_OPS[s]
                        nc.tensor.matmul(
                            qps[:, j, :],
                            id_sb[:],
                            xrt[:, ROT_IDX[a], :],
                            start=True,
                            stop=False,
                        )
                        nc.tensor.matmul(
                            qps[:, j, :],
                            id_sb[:],
                            xrt[:, ROT_IDX[bb], :],
                            start=False,
                            stop=True,
                        )
                    q2 = sqp.tile(
                        [128, 2, TILE_B], bf16, tag="q2", name="q2_t"
                    )
                    nc.scalar.activation(
                        q2[:], qps[:], mybir.ActivationFunctionType.Square
                    )
                    for j, s in enumerate(pair):
                        q2_tiles[s] = q2[:, j, :]
                # squares class (s=0): x_p^2 on ACT straight from SBUF
                x2 = x2p.tile([128, TILE_B], bf16, tag="x2", name="x2_t")
                nc.scalar.activation(
                    x2[:], xrt[:, 0, :], mybir.ActivationFunctionType.Square
                )
                return q2_tiles, x2

            # ramp order: identity first (tiny, gates the first
            # add-matmuls), then xc[0] (gates everything), then weights
            nc.sync.dma_start(id_sb[:], id_in[:])
            load_xc(0)
            load_xc(1)
            nc.sync.dma_start(w_sb[:], w_in[:])
            nc.sync.dma_start(b_sb[:], b_in[:])
            load_xc(2)
            trick = [None] * (N_TILES + 1)
            trick[0] = trick_phase(0)
            for t in range(N_TILES):
                load_xc(t + 3)
                xrt = xc_tiles[t]
                qrt = qc_tiles[t]
                trick[t + 1] = trick_phase(t + 1)
                q2_tiles, x2 = trick[t]

                # DVE product groups
                prod_views = {}  # class s -> AP of its product row
                for k, classes in enumerate(DVE_GROUPS):
                    m = len(classes)
                    p_t = prodp.tile(
                        [128, m, TILE_B], bf16, tag="prod" + str(m), name="p_t"
                    )
                    in0, in1 = rot_group_ap(xrt, classes)
                    nc.vector.tensor_mul(p_t[:], in0, in1)
                    for j, s in enumerate(classes):
                        prod_views[s] = p_t[:, j, :]

                # acc halves: even classes + linear -> partitions 0:64
                # (array cols 0-63), odd classes -> partitions 64:128
                acc = psump.tile([128, TILE_B], f32, name="acc")
                nc.tensor.matmul(
                    acc[0:64, :],
                    w_sb[:, 0:64],
                    xrt[:, 0, :],
                    start=True,
                    stop=False,
                    tile_position=(0, 0),
                )
                first_odd = True
                for s in range(65):
                    if s == 0:
                        rhs = x2[:]
                    elif s in q2_tiles:
                        rhs = q2_tiles[s]
                    elif s in prod_views:
                        rhs = prod_views[s]
                    else:
                        rhs = qrt[:, s - HOSTQ_CLASSES[0], :]
                    half = s % 2
                    blk = 1 + s
                    is_last_even = s == 64
                    is_last_odd = s == 63
                    nc.tensor.matmul(
                        acc[64 * half : 64 * half + 64, :],
                        w_sb[:, blk * 64 : (blk + 1) * 64],
                        rhs,
                        start=(half == 1 and first_odd),
                        stop=(is_last_even or is_last_odd),
                        tile_position=(0, 64 * half),
                    )
                    if half == 1:
                        first_odd = False

                # one ACT evacuates both PSUM halves (bias zero-padded on
                # the odd half); halves are summed host-side after gather
                o_t = outp.tile([128, TILE_B], f32, tag="o", name="o_t")
                nc.scalar.activation(
                    o_t[:],
                    acc[:],
                    mybir.ActivationFunctionType.Identity,
                    bias=b_sb[:, 0:1],
                )
                bs = slice(t * TILE_B, (t + 1) * TILE_B)
                nc.scalar.dma_start(out_ext[:, bs], o_t[:])

    _split_multiwaits(nc, mybir)

    # ---- per-core input maps ----
    in_maps = []
    for c in range(N_CORES):
        csl = slice(c * B_CORE, (c + 1) * B_CORE)
        cs = xall[:, :, csl]  # [128, N_ROT, 4096]
        xtiles = np.ascontiguousarray(
            cs.reshape(128, N_ROT, N_TILES, TILE_B).transpose(2, 0, 1, 3)
        )
        bias128 = np.zeros((128, 1), np.float32)
        bias128[:OUTPUT_DIM, 0] = bias
        m = {
            "xall": xtiles,
            "Wd": w_packed,
            "bias": bias128,
            "ident": np.ascontiguousarray(ident),
        }
        if N_HOSTQ:
            qs = q2h[:, :, csl]
            m["q2all"] = np.ascontiguousarray(
                qs.reshape(128, N_HOSTQ, N_TILES, TILE_B).transpose(2, 0, 1, 3)
            )
        in_maps.append(m)
    return nc, in_maps


def assemble(results):
    """Gather per-core outputs into the full [BATCH, 64] array (summing
    the even/odd PSUM halves host-side)."""
    outs = []
    for r in results:
        o = np.asarray(r["outT"], np.float32)  # [128, B_CORE]
        outs.append((o[0:OUTPUT_DIM] + o[OUTPUT_DIM:128]).T)
    return np.concatenate(outs, axis=0)


def kernel(x, W, b, indices_0, indices_1):
    from concourse.bass_utils import run_bass_kernel_spmd

    nc, in_maps = build(x, W, b)
    res = run_bass_kernel_spmd(nc, in_maps, list(range(N_CORES))).results
    return assemble(res)
